# revision 20
# baseline (speedup 1.0000x reference)
"""CASSI layer (mask -> shear-sum -> adjoint remask) as a Bass/Tile kernel
for 8 Trainium2 NeuronCores, pure data-parallel over the batch dim.

Per core: 2 batches x 4 row-blocks of [128 rows, 512*12].
  DVE   : in-place mask multiply + one strided tensor_reduce (shear sum)
  GPSIMD: X = mask * gather(Y)
  SyncE : HWDGE DMAs
Mask is recomputed on device from wr/wg/wb/wc (color bases passed as a
host-side constant table, input-independent).
"""

import numpy as np

M = 512
N = 512
L = 12
NT = 32
B = 16
NCORES = 8
BPC = B // NCORES          # batches per core
RB = M // 128              # row blocks per batch
NY = N + L - 1             # 523
NL = N * L                 # 6144
PAD = (L - 1) * L          # 132 zero pad on both sides of the masked row
FREE = NL + 2 * PAD        # 6408
NREP = N // NT             # 16

_BUILT = None


def _ensure_path():
    try:
        import concourse.bass  # noqa: F401
    except ImportError:
        import sys
        sys.path.insert(0, "/opt/trn_rl_repo")


def _color_bases():
    wl = np.linspace(420.0, 660.0, L).astype(np.float32) * np.float32(1e-9)

    def g(center_nm, sigma_nm):
        z = (wl - np.float32(center_nm * 1e-9)) / np.float32(sigma_nm * 1e-9)
        return np.exp(np.float32(-0.5) * z * z).astype(np.float32)

    fr = g(620.0, 40.0)
    fg = g(530.0, 40.0)
    fb = g(450.0, 40.0)
    fc = g(500.0, 60.0)
    return fr, fg, fb, fc


def _build_program():
    global _BUILT
    if _BUILT is not None:
        return _BUILT
    _ensure_path()
    import concourse.bacc as bacc
    import concourse.bass as bass
    import concourse.tile as tile
    from concourse import mybir

    F32 = mybir.dt.float32
    AP = bass.AP

    nc = bacc.Bacc("TRN2", target_bir_lowering=False, debug=False)

    x_d = nc.dram_tensor("x", [BPC, M, NL], F32, kind="ExternalInput")
    # wfb = w4 (128 cols) | fbases (48 cols), one load
    wfb_d = nc.dram_tensor("wfb", [128, 4 * NT + 4 * L], F32, kind="ExternalInput")
    xo_d = nc.dram_tensor("Xout", [BPC, M, NL], F32, kind="ExternalOutput")
    yo_d = nc.dram_tensor("Yout", [BPC, M, NY], F32, kind="ExternalOutput")

    with tile.TileContext(nc) as tc:
        with (
            tc.tile_pool(name="setup", bufs=1) as setup,
            tc.tile_pool(name="maskp", bufs=1) as maskp,
            tc.tile_pool(name="inp", bufs=1) as inpp,
            tc.tile_pool(name="xp", bufs=2) as xp,
            tc.tile_pool(name="yp", bufs=3) as yp,
        ):
            # ---------- mask setup, all 128 partitions at once ----------
            # w4/fbases arrive host-replicated to 128 partitions, so the mask
            # is computed in place with no cross-partition broadcast. The tiny
            # setup loads go first on the scalar queue, ahead of the big
            # input loads, so the mask is ready when in(0) lands.
            wfb = setup.tile([128, 4 * NT + 4 * L], F32)
            nc.scalar.dma_start(wfb[:], wfb_d[:])
            w4 = wfb[:, : 4 * NT]
            fbt = wfb[:, 4 * NT :]

            wt = setup.tile([128, NT], F32)
            wsum = AP(tensor=w4.tensor, offset=w4.offset,
                      ap=[w4.ap[0], [1, NT], [NT, 4]])
            nc.vector.tensor_reduce(
                wt[:], wsum, axis=mybir.AxisListType.X, op=mybir.AluOpType.add
            )
            rec = setup.tile([128, NT], F32)
            nc.vector.reciprocal(rec[:], wt[:])

            # mask[p, q*L + l] = sum_c w4[p, c*NT+q] * fbases[p, c*L+l] / wt
            mask = maskp.tile([128, NT * L], F32)
            tmp = setup.tile([128, NT * L], F32)
            w4f = w4
            fbf = fbt
            accf = mask[:]
            tmpf = tmp[:]
            acc3 = AP(tensor=accf.tensor, offset=accf.offset,
                      ap=[accf.ap[0], [L, NT], [1, L]])
            tmp3 = AP(tensor=tmpf.tensor, offset=tmpf.offset,
                      ap=[tmpf.ap[0], [L, NT], [1, L]])
            for c in range(4):
                w_b = AP(tensor=w4f.tensor, offset=w4f.offset + c * NT,
                         ap=[w4f.ap[0], [1, NT], [0, L]])
                f_b = AP(tensor=fbf.tensor, offset=fbf.offset + c * L,
                         ap=[fbf.ap[0], [0, NT], [1, L]])
                dst = acc3 if c == 0 else tmp3
                nc.vector.tensor_mul(dst, w_b, f_b)
                if c > 0:
                    nc.vector.tensor_add(acc3, acc3, tmp3)
            recf = rec[:]
            rec_b = AP(tensor=recf.tensor, offset=recf.offset,
                       ap=[recf.ap[0], [1, NT], [0, L]])
            nc.vector.tensor_mul(acc3, acc3, rec_b)
            maskf = mask[:]
            # broadcast view matching a [NREP, NT, L] tiling of a full row
            mask_b = AP(tensor=maskf.tensor, offset=maskf.offset,
                        ap=[maskf.ap[0], [0, NREP], [L, NT], [1, L]])

            # ---------- main loop ----------
            tin = [
                inpp.tile([128, FREE], F32, tag=f"tin{i}", name=f"tin{i}")
                for i in range(2)
            ]
            for t in tin:
                nc.vector.memset(t[:, 0:PAD], 0.0)
                nc.vector.memset(t[:, PAD + NL :], 0.0)

            HREP = NREP // 2
            for it in range(BPC * RB):
                bi, rb = divmod(it, RB)
                t = tin[it % 2]
                tf = t[:]
                rows = slice(rb * 128, (rb + 1) * 128)
                # loads on the scalar (ACT) HWDGE queue: never behind stores.
                # Iteration 0 is latency-critical (nothing to hide behind), so
                # split its load + mask-mul in column halves to start earlier.
                nsplit = 2 if it == 0 else 1
                step = NL // nsplit
                rstep = NREP // nsplit
                for s in range(nsplit):
                    nc.scalar.dma_start(
                        t[:, PAD + s * step : PAD + (s + 1) * step],
                        x_d[bi, rows, s * step : (s + 1) * step],
                    )
                    # op1: masked = x * mask (in place), dims [P, rstep, NT, L]
                    mid3 = AP(tensor=tf.tensor, offset=tf.offset + PAD + s * step,
                              ap=[tf.ap[0], [NT * L, rstep], [L, NT], [1, L]])
                    m_s = AP(tensor=mask_b.tensor, offset=mask_b.offset,
                             ap=[mask_b.ap[0], [0, rstep], [L, NT], [1, L]])
                    nc.vector.tensor_mul(mid3, mid3, m_s)

                # op2: Y[n'] = sum_i masked[PAD + L*n' - (L-1)*i]
                y = yp.tile([128, NY], F32)
                red = AP(tensor=tf.tensor, offset=tf.offset + (L - 1),
                         ap=[tf.ap[0], [L, NY], [L - 1, L]])
                nc.vector.tensor_reduce(
                    y[:], red, axis=mybir.AxisListType.X, op=mybir.AluOpType.add
                )

                # op3: X[n, l] = mask[n, l] * Y[n + l] on GPSIMD, split in
                # halves so stores can start early and blocking windows shrink
                xt = xp.tile([128, NL], F32)
                xtf = xt[:]
                yf = y[:]
                mb = mask_b
                # Last iteration: nothing left for GPSIMD to overlap and the
                # DVE is ~2x faster per element, so it runs everything there
                # in quarters with the stores streaming out behind it.
                last = it == BPC * RB - 1
                if last:
                    pieces = [(nc.vector, r, 4) for r in range(0, NREP, 4)]
                else:
                    pieces = [(nc.gpsimd, 0, HREP), (nc.gpsimd, HREP, HREP)]
                for eng, r0, nr in pieces:
                    xt3 = AP(tensor=xtf.tensor,
                             offset=xtf.offset + r0 * NT * L,
                             ap=[xtf.ap[0], [NT * L, nr], [L, NT], [1, L]])
                    m_h = AP(tensor=mb.tensor, offset=mb.offset,
                             ap=[mb.ap[0], [0, nr], [L, NT], [1, L]])
                    y_g = AP(tensor=yf.tensor, offset=yf.offset + r0 * NT,
                             ap=[yf.ap[0], [NT, nr], [1, NT], [1, L]])
                    eng.tensor_tensor(xt3, m_h, y_g, mybir.AluOpType.mult)
                    nc.sync.dma_start(
                        xo_d[bi, rows, r0 * NT * L : (r0 + nr) * NT * L],
                        xt[:, r0 * NT * L : (r0 + nr) * NT * L],
                    )
                nc.sync.dma_start(yo_d[bi, rows, :], y[:])

    nc.compile()
    _BUILT = nc
    return nc


def _in_maps(inputs, wr, wg, wb, wc):
    inputs = np.ascontiguousarray(np.asarray(inputs, dtype=np.float32))
    w4 = np.concatenate(
        [np.asarray(w, dtype=np.float32).reshape(NT, NT) for w in (wr, wg, wb, wc)],
        axis=1,
    )
    w4 = np.tile(w4, (128 // NT, 1))
    fr, fg, fb, fc = _color_bases()
    fbases = np.tile(np.concatenate([fr, fg, fb, fc])[None, :], (128, 1))
    wfb = np.ascontiguousarray(
        np.concatenate([w4, fbases], axis=1), dtype=np.float32
    )
    x = inputs.reshape(B, M, NL)
    return [
        {"x": x[c * BPC : (c + 1) * BPC], "wfb": wfb} for c in range(NCORES)
    ]


def _assemble(results):
    X = np.concatenate([r["Xout"] for r in results], axis=0)
    Y = np.concatenate([r["Yout"] for r in results], axis=0)
    return (
        X.reshape(B, M, N, L),
        Y.reshape(B, M, NY, 1),
    )


def _run(inputs, wr, wg, wb, wc, trace=False):
    _ensure_path()
    import os

    if not trace:
        # the NTFF trace path needs hooks this image may not have; make sure
        # a stray BASS_TRACE env var can't route us into it
        os.environ["BASS_NEVER_TRACE"] = "1"
    else:
        os.environ.pop("BASS_NEVER_TRACE", None)
    from concourse.bass_utils import run_bass_kernel_spmd

    nc = _build_program()
    res = run_bass_kernel_spmd(
        nc, _in_maps(inputs, wr, wg, wb, wc), list(range(NCORES)), trace=trace
    )
    return _assemble(res.results), res


def kernel(inputs, wr, wg, wb, wc):
    (X, Y), _ = _run(inputs, wr, wg, wb, wc)
    return X, Y
